# revision 1
# baseline (speedup 1.0000x reference)
"""DbrxExperts MoE kernel for 8 Trainium2 NeuronCores (expert-parallel).

Problem: E=16 experts, top_k=4, H=2048, F=4096, T=64 tokens.
out = sum_e r[:, e] * (silu(x @ w1_e.T) * (x @ v1_e.T)) @ w2_e
with r = scatter-add of top_weights into dense [T, E].

Strategy: expert-parallel across 8 cores (2 experts per core). Each core
streams its 2 experts' weights (bf16-cast on host: halves HBM traffic;
fp32 PSUM accumulation keeps rel-err ~4e-3) and computes a partial
output [T, H]; host sums the 8 partials. Routing weights are folded into
the `up` projection drain, so the down-projection accumulates both local
experts directly in PSUM.

Weight layouts are pre-swizzled on the host so every weight DMA is a
fully contiguous 2 MiB transfer of shape [128, 8192] bf16.
"""

import os
import sys
import types

import numpy as np
import ml_dtypes

BF16 = ml_dtypes.bfloat16

E, TOPK, H, F = 16, 4, 2048, 4096
T = 64
N_CORES = 8
EPC = E // N_CORES          # experts per core = 2
KT = H // 128               # 16 k-tiles of 128 over H
FCH = 8                     # f-chunks of 512 over F
FC = F // FCH               # 512
NCH = EPC * FCH             # 16 weight chunks per core per matrix


def _ensure_axon_hooks():
    """antenv.axon_hooks is missing from the stub antenv shipped in some
    containers; run_bass_kernel_spmd(trace=True) imports it under axon.
    Register the ctypes NTFF hook when libaxon_pjrt.so is present, else a
    None-returning stub so tracing degrades instead of crashing."""
    try:
        import antenv.axon_hooks  # noqa: F401
        return
    except ImportError:
        pass
    try:
        import antenv
    except ImportError:
        return
    mod = types.ModuleType("antenv.axon_hooks")
    _hook = [None]
    mod.set_axon_ntff_profile_hook = lambda h: _hook.__setitem__(0, h)
    mod.get_axon_ntff_profile_hook = lambda: _hook[0]
    sys.modules["antenv.axon_hooks"] = mod
    antenv.axon_hooks = mod
    try:
        from trn_agent_boot.trn_boot import _ntff_profile_via_ctypes

        so_path = "/opt/axon/libaxon_pjrt.so"
        if os.path.exists(so_path):
            h = _ntff_profile_via_ctypes(so_path)
            if h is not None:
                mod.set_axon_ntff_profile_hook(h)
    except Exception:
        pass


def _build_nc():
    import concourse.mybir as mybir
    import concourse.tile as tile
    from concourse import bacc

    f32 = mybir.dt.float32
    bf16 = mybir.dt.bfloat16

    nc = bacc.Bacc("TRN2", debug=False, num_devices=N_CORES)
    xt_d = nc.dram_tensor("xt", [1 + EPC, 128, KT * T], bf16, kind="ExternalInput")
    w1_d = nc.dram_tensor("w1t", [NCH, 128, KT * FC], bf16, kind="ExternalInput")
    v1_d = nc.dram_tensor("v1t", [NCH, 128, KT * FC], bf16, kind="ExternalInput")
    w2_d = nc.dram_tensor("w2s", [NCH, 128, 4 * H], bf16, kind="ExternalInput")
    out_d = nc.dram_tensor("out", [T, H], f32, kind="ExternalOutput")

    act = mybir.ActivationFunctionType

    with tile.TileContext(nc) as tc:
        with (
            tc.tile_pool(name="const", bufs=1) as const_pool,
            tc.tile_pool(name="w1", bufs=3) as w1_pool,
            tc.tile_pool(name="v1", bufs=3) as v1_pool,
            tc.tile_pool(name="w2", bufs=4) as w2_pool,
            tc.tile_pool(name="acts", bufs=4) as acts_pool,
            tc.tile_pool(name="ps_gate", bufs=2, space="PSUM") as ps_gate,
            tc.tile_pool(name="ps_up", bufs=2, space="PSUM") as ps_up,
            tc.tile_pool(name="ps_tp", bufs=2, space="PSUM") as ps_tp,
            tc.tile_pool(name="ps_down", bufs=1, space="PSUM") as ps_down,
        ):
            # constants / whole-kernel tiles (scalar HWDGE queue, so they
            # don't queue behind the weight stream on the sync queue)
            xt_sb = const_pool.tile([128, KT * T], bf16)
            nc.scalar.dma_start(xt_sb[:], xt_d[0])
            xtu_sb = []
            for e in range(EPC):
                t_ = const_pool.tile([128, KT * T], bf16, tag=f"xtu{e}")
                nc.scalar.dma_start(t_[:], xt_d[1 + e])
                xtu_sb.append(t_)
            ident = const_pool.tile([64, 64], bf16)
            from concourse.masks import make_identity

            make_identity(nc, ident)

            # persistent down-projection accumulator:
            # [0:64, 0:1024] = hid 0..1023, [64:128, 0:1024] = hid 1024..2047
            down_ps = ps_down.tile([128, 1024], mybir.dt.float32)

            HKT = KT // 2  # k-tiles per half-chunk DMA

            def piece(e, w1a, w1b, v1a, v1b, w2c, fo, fw, first, last):
                """Process f-range [fo, fo+fw) of the current 512-wide chunk."""
                gate_ps = ps_gate.tile([T, fw], mybir.dt.float32, tag="gate")
                up_ps = ps_up.tile([T, fw], mybir.dt.float32, tag="up")
                for i in range(KT):
                    wsrc = w1a if i < HKT else w1b
                    lo = (i % HKT) * FC + fo
                    nc.tensor.matmul(
                        gate_ps[:],
                        xt_sb[:, i * T : (i + 1) * T],
                        wsrc[:, lo : lo + fw],
                        start=(i == 0),
                        stop=(i == KT - 1),
                    )
                for i in range(KT):
                    vsrc = v1a if i < HKT else v1b
                    lo = (i % HKT) * FC + fo
                    nc.tensor.matmul(
                        up_ps[:],
                        xtu_sb[e][:, i * T : (i + 1) * T],
                        vsrc[:, lo : lo + fw],
                        start=(i == 0),
                        stop=(i == KT - 1),
                    )

                gate_s = acts_pool.tile([T, fw], bf16, tag="gate_s")
                nc.scalar.activation(gate_s[:], gate_ps[:], act.Silu)
                h = acts_pool.tile([T, fw], bf16, tag="h")
                nc.vector.tensor_mul(h[:], gate_s[:], up_ps[:])

                # transpose h [64, fw] -> hT tiles [128, 64] via PE
                ntp = fw // 128
                tp_ps = ps_tp.tile([128, ntp * T], bf16, tag="tp")
                for j in range(ntp):
                    nc.tensor.transpose(
                        tp_ps[:, j * T : (j + 1) * T],
                        h[:, j * 128 : (j + 1) * 128],
                        ident[:],
                    )
                hT = acts_pool.tile([128, ntp * T], bf16, tag="hT")
                nc.vector.tensor_copy(hT[:], tp_ps[:])

                for j in range(ntp):
                    jg = (fo + j * 128) // 128  # f-tile index within chunk
                    for q in range(4):
                        if q < 2:
                            dst = down_ps[0:T, q * 512 : (q + 1) * 512]
                        else:
                            dst = down_ps[64 : 64 + T, (q - 2) * 512 : (q - 1) * 512]
                        nc.tensor.matmul(
                            dst,
                            hT[:, j * T : (j + 1) * T],
                            w2c[:, jg * H + q * 512 : jg * H + (q + 1) * 512],
                            start=(first and j == 0),
                            stop=(last and j == ntp - 1),
                        )

            for e in range(EPC):
                for c in range(FCH):
                    ci = e * FCH + c
                    # half-split weight tiles: PE can start on half A while
                    # half B is still in flight. w1/w2 issue on the sync
                    # HWDGE queue, v1 on the scalar queue (parallel rings).
                    w1a = w1_pool.tile([128, HKT * FC], bf16, tag="w1a")
                    nc.sync.dma_start(w1a[:], w1_d[ci, :, : HKT * FC])
                    w1b = w1_pool.tile([128, HKT * FC], bf16, tag="w1b")
                    nc.sync.dma_start(w1b[:], w1_d[ci, :, HKT * FC :])
                    v1a = v1_pool.tile([128, HKT * FC], bf16, tag="v1a")
                    nc.scalar.dma_start(v1a[:], v1_d[ci, :, : HKT * FC])
                    v1b = v1_pool.tile([128, HKT * FC], bf16, tag="v1b")
                    nc.scalar.dma_start(v1b[:], v1_d[ci, :, HKT * FC :])
                    w2c = w2_pool.tile([128, 4 * H], bf16, tag="w2c")
                    nc.sync.dma_start(w2c[:], w2_d[ci])

                    first = e == 0 and c == 0
                    if e == EPC - 1 and c == FCH - 1:
                        # split the final chunk into two pieces: shortens the
                        # end-of-kernel chain without doubling PE issue count
                        for s_ in range(2):
                            piece(
                                e, w1a, w1b, v1a, v1b, w2c,
                                s_ * 256, 256,
                                first=False, last=(s_ == 1),
                            )
                    else:
                        piece(e, w1a, w1b, v1a, v1b, w2c, 0, FC, first, False)

            # final drain: two engines in parallel, DMA per half as soon
            # as its copy lands (DVE does hid 0-1023, ACT does 1024-2047)
            out_sb = const_pool.tile([128, 1024], mybir.dt.float32)
            nc.vector.tensor_copy(out_sb[0:T], down_ps[0:T])
            nc.sync.dma_start(out_d[:, 0:1024], out_sb[0:T])
            nc.scalar.activation(
                out_sb[64 : 64 + T], down_ps[64 : 64 + T], act.Copy
            )
            nc.scalar.dma_start(out_d[:, 1024:2048], out_sb[64 : 64 + T])

    nc.compile()
    return nc


_NC_CACHE = None


def _get_nc():
    global _NC_CACHE
    if _NC_CACHE is None:
        _NC_CACHE = _build_nc()
    return _NC_CACHE


def _swizzle_ffn(wt):
    """[H, F] (h, f) -> [FCH, 128, KT*FC] so chunk c is a contiguous
    [128, 8192] block with [p, i*FC + f] = wt[i*128 + p, c*FC + f]."""
    a = wt.reshape(KT, 128, FCH, FC)          # (i, p, c, f)
    return np.ascontiguousarray(a.transpose(2, 1, 0, 3)).reshape(FCH, 128, KT * FC)


def _swizzle_down(w2e):
    """[F, H] (f, hid) -> [FCH, 128, 4*H] so chunk c is contiguous
    [128, 8192] with [p, j*H + hid] = w2e[c*FC + j*128 + p, hid]."""
    a = w2e.reshape(FCH, 4, 128, H)           # (c, j, p, hid)
    return np.ascontiguousarray(a.transpose(0, 2, 1, 3)).reshape(FCH, 128, 4 * H)


def kernel(x, weights, top_weights, top_experts, w1, v1, w2):
    _ensure_axon_hooks()
    from concourse.bass_utils import run_bass_kernel_spmd

    x = np.asarray(x, dtype=np.float32).reshape(T, H)
    top_weights = np.asarray(top_weights, dtype=np.float32)
    top_experts = np.asarray(top_experts).astype(np.int64)
    w1 = np.asarray(w1, dtype=np.float32).reshape(E, F, H)
    v1 = np.asarray(v1, dtype=np.float32).reshape(E, F, H)
    w2 = np.asarray(w2, dtype=np.float32).reshape(E, F, H)

    # dense routing weights [T, E] (scatter-ADD: duplicate experts sum)
    r = np.zeros((T, E), np.float32)
    np.add.at(r, (np.arange(T)[:, None], top_experts), top_weights)

    # x transposed/swizzled: [128, KT*T] with [p, i*T + t] = x[t, i*128 + p]
    def swz_x(a):
        return np.ascontiguousarray(
            a.T.reshape(KT, 128, T).transpose(1, 0, 2)
        ).reshape(128, KT * T).astype(BF16)

    xt = swz_x(x)

    in_maps = []
    for core in range(N_CORES):
        es = [core * EPC + k for k in range(EPC)]
        w1t = np.concatenate(
            [_swizzle_ffn(w1[e].T.astype(BF16)) for e in es], axis=0
        )
        v1t = np.concatenate(
            [_swizzle_ffn(v1[e].T.astype(BF16)) for e in es], axis=0
        )
        w2s = np.concatenate(
            [_swizzle_down(w2[e].astype(BF16)) for e in es], axis=0
        )
        # plane 0: x for the gate path; planes 1+k: r_e-scaled x for the
        # up path (folds the routing weight into the matmul operand)
        xt_planes = np.stack(
            [xt] + [swz_x(x * r[:, ee : ee + 1]) for ee in es], axis=0
        )
        in_maps.append(
            {
                "xt": xt_planes,
                "w1t": w1t,
                "v1t": v1t,
                "w2s": w2s,
            }
        )

    nc = _get_nc()
    res = run_bass_kernel_spmd(nc, in_maps, core_ids=list(range(N_CORES)))
    out = np.zeros((T, H), np.float32)
    for c in range(N_CORES):
        out += res.results[c]["out"]
    return out.reshape(64, 1, H)



# revision 3
# speedup vs baseline: 1.3806x; 1.3806x over previous
"""DbrxExperts MoE kernel for 8 Trainium2 NeuronCores (expert-parallel).

Problem: E=16 experts, top_k=4, H=2048, F=4096, T=64 tokens.
out = sum_e r[:, e] * (silu(x @ w1_e.T) * (x @ v1_e.T)) @ w2_e
with r = scatter-add of top_weights into dense [T, E].

Strategy: expert-parallel across 8 cores (2 experts per core), with the
two experts PAIRED across the PE array's column halves so both stream
concurrently (M=64 alone wastes half the 128-wide array):
  - gate/up matmuls: expert 0 writes PSUM partitions 0-63, expert 1
    writes partitions 64-127; consecutive matmuls alternate column
    groups, so the PE executes two M=64 matmuls at once.
  - w1/v1 are stored fp8e3 (e3m4) with a global power-of-2 scale folded
    exactly into the bf16 x operand planes (halves their HBM traffic;
    measured rel-err 1.65e-2 < 2e-2). w2 stays bf16.
  - h [128(t: e0|e1), 512] is transposed in full 128x128 blocks; the
    transposed tile has each expert's tokens in separate 64-col slices,
    used directly as the down-projection stationary operands.

Weight layouts are pre-swizzled on the host so every weight DMA is a
fully contiguous >=1 MiB transfer.
"""

import os
import sys
import types

import numpy as np
import ml_dtypes

BF16 = ml_dtypes.bfloat16
F8E3 = ml_dtypes.float8_e3m4

E, TOPK, H, F = 16, 4, 2048, 4096
T = 64
N_CORES = 8
EPC = E // N_CORES          # experts per core = 2
KT = H // 128               # 16 k-tiles of 128 over H
FCH = 8                     # f-chunks of 512 over F
FC = F // FCH               # 512
F8MAX = 15.5


def _ensure_axon_hooks():
    """antenv.axon_hooks is missing from the stub antenv shipped in some
    containers; run_bass_kernel_spmd(trace=True) imports it under axon.
    Register the ctypes NTFF hook when libaxon_pjrt.so is present, else a
    None-returning stub so tracing degrades instead of crashing."""
    try:
        import antenv.axon_hooks  # noqa: F401
        return
    except ImportError:
        pass
    try:
        import antenv
    except ImportError:
        return
    mod = types.ModuleType("antenv.axon_hooks")
    _hook = [None]
    mod.set_axon_ntff_profile_hook = lambda h: _hook.__setitem__(0, h)
    mod.get_axon_ntff_profile_hook = lambda: _hook[0]
    sys.modules["antenv.axon_hooks"] = mod
    antenv.axon_hooks = mod
    try:
        from trn_agent_boot.trn_boot import _ntff_profile_via_ctypes

        so_path = "/opt/axon/libaxon_pjrt.so"
        if os.path.exists(so_path):
            h = _ntff_profile_via_ctypes(so_path)
            if h is not None:
                mod.set_axon_ntff_profile_hook(h)
    except Exception:
        pass


def _build_nc():
    import concourse.mybir as mybir
    import concourse.tile as tile
    from concourse import bacc
    from concourse.masks import make_identity

    f32 = mybir.dt.float32
    bf16 = mybir.dt.bfloat16
    f8 = mybir.dt.float8e3

    nc = bacc.Bacc("TRN2", debug=False, num_devices=N_CORES)
    xt_d = nc.dram_tensor("xt", [1 + EPC, 128, KT * T], bf16, kind="ExternalInput")
    w1_d = nc.dram_tensor("w1t", [FCH, 128, EPC * KT * FC], f8, kind="ExternalInput")
    v1_d = nc.dram_tensor("v1t", [FCH, 128, EPC * KT * FC], f8, kind="ExternalInput")
    w2_d = nc.dram_tensor("w2s", [FCH, 128, EPC * 4 * H], bf16, kind="ExternalInput")
    out_d = nc.dram_tensor("out", [T, H], f32, kind="ExternalOutput")

    act = mybir.ActivationFunctionType
    EW = KT * FC  # per-expert free width in w1/v1 chunk tiles (8192)

    with tile.TileContext(nc) as tc:
        with (
            tc.tile_pool(name="const", bufs=1) as const_pool,
            tc.tile_pool(name="w1", bufs=3) as w1_pool,
            tc.tile_pool(name="v1", bufs=3) as v1_pool,
            tc.tile_pool(name="w2", bufs=2) as w2_pool,
            tc.tile_pool(name="acts", bufs=4) as acts_pool,
            tc.tile_pool(name="ps_gate", bufs=2, space="PSUM") as ps_gate,
            tc.tile_pool(name="ps_up", bufs=2, space="PSUM") as ps_up,
            tc.tile_pool(name="ps_tp", bufs=2, space="PSUM") as ps_tp,
            tc.tile_pool(name="ps_down", bufs=1, space="PSUM") as ps_down,
        ):
            # constants (scalar HWDGE ring so they don't queue behind weights)
            xg_sb = const_pool.tile([128, KT * T], bf16)
            nc.scalar.dma_start(xg_sb[:], xt_d[0])
            xu_sb = []
            for e in range(EPC):
                t_ = const_pool.tile([128, KT * T], bf16, tag=f"xu{e}")
                nc.scalar.dma_start(t_[:], xt_d[1 + e])
                xu_sb.append(t_)
            ident = const_pool.tile([128, 128], bf16)
            make_identity(nc, ident)

            # persistent down-projection accumulator:
            # [0:64, :] = hid 0..1023, [64:128, :] = hid 1024..2047
            down_ps = ps_down.tile([128, 1024], mybir.dt.float32)

            for c in range(FCH):
                w1c = w1_pool.tile([128, EPC * EW], f8, tag="w1c")
                v1c = v1_pool.tile([128, EPC * EW], f8, tag="v1c")
                w2c = w2_pool.tile([128, EPC * 4 * H], bf16, tag="w2c")
                # ring balance: sync = w1(2.1M) + w2_e0(2.1M); scalar =
                # v1(2.1M) + w2_e1(2.1M)
                nc.sync.dma_start(w1c[:, 0:EW], w1_d[c, :, 0:EW])
                nc.sync.dma_start(w1c[:, EW:], w1_d[c, :, EW:])
                nc.scalar.dma_start(v1c[:, 0:EW], v1_d[c, :, 0:EW])
                nc.scalar.dma_start(v1c[:, EW:], v1_d[c, :, EW:])
                nc.sync.dma_start(w2c[:, 0 : 4 * H], w2_d[c, :, 0 : 4 * H])
                nc.scalar.dma_start(w2c[:, 4 * H :], w2_d[c, :, 4 * H :])

                gate_ps = ps_gate.tile([128, FC], mybir.dt.float32, tag="gate")
                up_ps = ps_up.tile([128, FC], mybir.dt.float32, tag="up")
                # gate: both experts share the xg stationary; alternate
                # column groups (out partitions 0-63 / 64-127) so the two
                # M=64 matmuls run concurrently in the PE array.
                for i in range(KT):
                    for e in range(EPC):
                        nc.tensor.matmul(
                            gate_ps[64 * e : 64 * e + T, :],
                            xg_sb[:, i * T : (i + 1) * T],
                            w1c[:, e * EW + i * FC : e * EW + (i + 1) * FC],
                            start=(i == 0),
                            stop=(i == KT - 1),
                        )
                for i in range(KT):
                    for e in range(EPC):
                        nc.tensor.matmul(
                            up_ps[64 * e : 64 * e + T, :],
                            xu_sb[e][:, i * T : (i + 1) * T],
                            v1c[:, e * EW + i * FC : e * EW + (i + 1) * FC],
                            start=(i == 0),
                            stop=(i == KT - 1),
                        )

                gate_s = acts_pool.tile([128, FC], bf16, tag="gate_s")
                nc.scalar.activation(gate_s[:], gate_ps[:], act.Silu)
                h = acts_pool.tile([128, FC], bf16, tag="h")
                nc.vector.tensor_mul(h[:], gate_s[:], up_ps[:])

                # transpose h in full 128x128 blocks: block j becomes
                # [128 f, 128 t] with e0 tokens in cols 0-63, e1 in 64-127
                tp_ps = ps_tp.tile([128, 4 * 128], bf16, tag="tp")
                for j in range(4):
                    nc.tensor.transpose(
                        tp_ps[:, j * 128 : (j + 1) * 128],
                        h[:, j * 128 : (j + 1) * 128],
                        ident[:],
                    )
                hT = acts_pool.tile([128, 4 * 128], bf16, tag="hT")
                nc.vector.tensor_copy(hT[:], tp_ps[:])

                for j in range(4):
                    for e in range(EPC):
                        st = hT[:, j * 128 + 64 * e : j * 128 + 64 * e + T]
                        wb = e * 4 * H + j * H
                        first = c == 0 and j == 0 and e == 0
                        last = c == FCH - 1 and j == 3 and e == EPC - 1
                        # hid quarters q0..q3; order (0,2,1,3) alternates
                        # column groups for PE concurrency
                        for q in (0, 2, 1, 3):
                            if q < 2:
                                dst = down_ps[0:T, q * 512 : (q + 1) * 512]
                            else:
                                dst = down_ps[64 : 64 + T, (q - 2) * 512 : (q - 1) * 512]
                            nc.tensor.matmul(
                                dst,
                                st,
                                w2c[:, wb + q * 512 : wb + (q + 1) * 512],
                                start=first,
                                stop=last,
                            )

            # final drain: two engines in parallel, DMA per half as soon
            # as its copy lands (DVE does hid 0-1023, ACT does 1024-2047)
            out_sb = const_pool.tile([128, 1024], mybir.dt.float32)
            nc.vector.tensor_copy(out_sb[0:T], down_ps[0:T])
            nc.sync.dma_start(out_d[:, 0:1024], out_sb[0:T])
            nc.scalar.activation(
                out_sb[64 : 64 + T], down_ps[64 : 64 + T], act.Copy
            )
            nc.scalar.dma_start(out_d[:, 1024:2048], out_sb[64 : 64 + T])

    nc.compile()
    return nc


_NC_CACHE = None


def _get_nc():
    global _NC_CACHE
    if _NC_CACHE is None:
        _NC_CACHE = _build_nc()
    return _NC_CACHE


def _swizzle_ffn(wt):
    """[H, F] (h, f) -> [FCH, 128, KT*FC] so chunk c is a contiguous
    [128, 8192] block with [p, i*FC + f] = wt[i*128 + p, c*FC + f]."""
    a = wt.reshape(KT, 128, FCH, FC)          # (i, p, c, f)
    return np.ascontiguousarray(a.transpose(2, 1, 0, 3)).reshape(FCH, 128, KT * FC)


def _swizzle_down(w2e):
    """[F, H] (f, hid) -> [FCH, 128, 4*H] so chunk c is contiguous
    [128, 8192] with [p, j*H + hid] = w2e[c*FC + j*128 + p, hid]."""
    a = w2e.reshape(FCH, 4, 128, H)           # (c, j, p, hid)
    return np.ascontiguousarray(a.transpose(0, 2, 1, 3)).reshape(FCH, 128, 4 * H)


def _pair(chunks):
    """stack 2 per-expert [FCH, 128, W] arrays -> [FCH, 128, 2*W]"""
    a = np.stack(chunks, axis=2)              # (c, p, e, w) after transpose
    c, p, e_, w = a.shape[0], a.shape[1], a.shape[2], a.shape[3]
    return np.ascontiguousarray(a).reshape(c, p, e_ * w)


def _pow2_scale(amax):
    return int(np.floor(np.log2(F8MAX * 0.98 / amax)))


def kernel(x, weights, top_weights, top_experts, w1, v1, w2):
    _ensure_axon_hooks()
    from concourse.bass_utils import run_bass_kernel_spmd

    x = np.asarray(x, dtype=np.float32).reshape(T, H)
    top_weights = np.asarray(top_weights, dtype=np.float32)
    top_experts = np.asarray(top_experts).astype(np.int64)
    w1 = np.asarray(w1, dtype=np.float32).reshape(E, F, H)
    v1 = np.asarray(v1, dtype=np.float32).reshape(E, F, H)
    w2 = np.asarray(w2, dtype=np.float32).reshape(E, F, H)

    # dense routing weights [T, E] (scatter-ADD: duplicate experts sum)
    r = np.zeros((T, E), np.float32)
    np.add.at(r, (np.arange(T)[:, None], top_experts), top_weights)

    # global power-of-2 scales for the fp8 weights; folded exactly into
    # the bf16 x operand planes (exponent shift only)
    k1 = _pow2_scale(np.abs(w1).max())
    kv = _pow2_scale(np.abs(v1).max())
    w1q = (w1 * np.float32(2.0**k1)).astype(F8E3)
    v1q = (v1 * np.float32(2.0**kv)).astype(F8E3)

    # x transposed/swizzled: [128, KT*T] with [p, i*T + t] = a[t, i*128 + p]
    def swz_x(a):
        return np.ascontiguousarray(
            a.T.reshape(KT, 128, T).transpose(1, 0, 2)
        ).reshape(128, KT * T).astype(BF16)

    xg = swz_x(x * np.float32(2.0**-k1))

    in_maps = []
    for core in range(N_CORES):
        es = [core * EPC + k for k in range(EPC)]
        w1t = _pair([_swizzle_ffn(w1q[e].T) for e in es])
        v1t = _pair([_swizzle_ffn(v1q[e].T) for e in es])
        w2s = _pair([_swizzle_down(w2[e].astype(BF16)) for e in es])
        # plane 0: x*2^-k1 for the gate path; planes 1+k: r_e-scaled x
        # (times 2^-kv) for the up path
        xt_planes = np.stack(
            [xg]
            + [swz_x(x * (r[:, ee : ee + 1] * np.float32(2.0**-kv))) for ee in es],
            axis=0,
        )
        in_maps.append(
            {
                "xt": xt_planes,
                "w1t": w1t,
                "v1t": v1t,
                "w2s": w2s,
            }
        )

    nc = _get_nc()
    res = run_bass_kernel_spmd(nc, in_maps, core_ids=list(range(N_CORES)))
    out = np.zeros((T, H), np.float32)
    for c in range(N_CORES):
        out += res.results[c]["out"]
    return out.reshape(64, 1, H)


# revision 4
# speedup vs baseline: 1.6989x; 1.2306x over previous
"""DbrxExperts MoE kernel for 8 Trainium2 NeuronCores (expert-parallel).

Problem: E=16 experts, top_k=4, H=2048, F=4096, T=64 tokens.
out = sum_e r[:, e] * (silu(x @ w1_e.T) * (x @ v1_e.T)) @ w2_e
with r = scatter-add of top_weights into dense [T, E].

Strategy: expert-parallel across 8 cores (2 experts per core), with the
two experts PAIRED across the PE array's column halves so both stream
concurrently (M=64 alone wastes half the 128-wide array):
  - gate/up matmuls: expert 0 writes PSUM partitions 0-63, expert 1
    writes partitions 64-127; consecutive matmuls alternate column
    groups, so the PE executes two M=64 matmuls at once.
  - w1/v1 are stored fp8e3 (e3m4) with a global power-of-2 scale folded
    exactly into the bf16 x operand planes; w2 is fp8e3 for f-chunks 4-7
    (scale folded into the hT drain as an exact pow2 tensor_scalar_mul)
    and bf16 for chunks 0-3. Measured rel-err 1.90e-2 < 2e-2.
  - h [128(t: e0|e1), 512] is transposed in full 128x128 blocks; the
    transposed tile has each expert's tokens in separate 64-col slices,
    used directly as the down-projection stationary operands.

Weight layouts are pre-swizzled on the host so weight DMAs are fully
contiguous >=1 MiB transfers (the last chunk's w2 is split in halves to
shorten the end-of-stream dependency chain).
"""

import os
import sys
import types

import numpy as np
import ml_dtypes

BF16 = ml_dtypes.bfloat16
F8E3 = ml_dtypes.float8_e3m4

E, TOPK, H, F = 16, 4, 2048, 4096
T = 64
N_CORES = 8
EPC = E // N_CORES          # experts per core = 2
KT = H // 128               # 16 k-tiles of 128 over H
FCH = 8                     # f-chunks of 512 over F
FC = F // FCH               # 512
NQ8 = 4                     # trailing f-chunks with fp8 w2 (chunks FCH-NQ8..FCH-1)
F8MAX = 15.5


def _ensure_axon_hooks():
    """antenv.axon_hooks is missing from the stub antenv shipped in some
    containers; run_bass_kernel_spmd(trace=True) imports it under axon.
    Register the ctypes NTFF hook when libaxon_pjrt.so is present, else a
    None-returning stub so tracing degrades instead of crashing."""
    try:
        import antenv.axon_hooks  # noqa: F401
        return
    except ImportError:
        pass
    try:
        import antenv
    except ImportError:
        return
    mod = types.ModuleType("antenv.axon_hooks")
    _hook = [None]
    mod.set_axon_ntff_profile_hook = lambda h: _hook.__setitem__(0, h)
    mod.get_axon_ntff_profile_hook = lambda: _hook[0]
    sys.modules["antenv.axon_hooks"] = mod
    antenv.axon_hooks = mod
    try:
        from trn_agent_boot.trn_boot import _ntff_profile_via_ctypes

        so_path = "/opt/axon/libaxon_pjrt.so"
        if os.path.exists(so_path):
            h = _ntff_profile_via_ctypes(so_path)
            if h is not None:
                mod.set_axon_ntff_profile_hook(h)
    except Exception:
        pass


def _build_nc(k2):
    import concourse.mybir as mybir
    import concourse.tile as tile
    from concourse import bacc
    from concourse.masks import make_identity

    f32 = mybir.dt.float32
    bf16 = mybir.dt.bfloat16
    f8 = mybir.dt.float8e3

    nc = bacc.Bacc("TRN2", debug=False, num_devices=N_CORES)
    xt_d = nc.dram_tensor("xt", [1 + EPC, 128, KT * T], bf16, kind="ExternalInput")
    w1_d = nc.dram_tensor("w1t", [FCH, 128, EPC * KT * FC], f8, kind="ExternalInput")
    v1_d = nc.dram_tensor("v1t", [FCH, 128, EPC * KT * FC], f8, kind="ExternalInput")
    w2_d = nc.dram_tensor(
        "w2s", [FCH - NQ8, 128, EPC * 4 * H], bf16, kind="ExternalInput"
    )
    w2q_d = nc.dram_tensor("w2q", [NQ8, 128, EPC * 4 * H], f8, kind="ExternalInput")
    out_d = nc.dram_tensor("out", [T, H], f32, kind="ExternalOutput")

    act = mybir.ActivationFunctionType
    EW = KT * FC  # per-expert free width in w1/v1 chunk tiles (8192)
    k2inv = float(2.0**-k2)

    with tile.TileContext(nc) as tc:
        with (
            tc.tile_pool(name="const", bufs=1) as const_pool,
            tc.tile_pool(name="w1", bufs=3) as w1_pool,
            tc.tile_pool(name="v1", bufs=3) as v1_pool,
            tc.tile_pool(name="w2", bufs=2) as w2_pool,
            tc.tile_pool(name="acts", bufs=4) as acts_pool,
            tc.tile_pool(name="ps_gate", bufs=2, space="PSUM") as ps_gate,
            tc.tile_pool(name="ps_up", bufs=2, space="PSUM") as ps_up,
            tc.tile_pool(name="ps_tp", bufs=2, space="PSUM") as ps_tp,
            tc.tile_pool(name="ps_down", bufs=1, space="PSUM") as ps_down,
        ):
            # chunk 0 weight DMAs first so both HWDGE rings start pumping
            # immediately; consts follow on the scalar ring.
            wtiles = {}

            def issue_chunk_dma(c):
                fp8w2 = c >= FCH - NQ8
                w1c = w1_pool.tile([128, EPC * EW], f8, tag="w1c")
                v1c = v1_pool.tile([128, EPC * EW], f8, tag="v1c")
                w2c = w2_pool.tile(
                    [128, EPC * 4 * H], f8 if fp8w2 else bf16, tag="w2c"
                )
                nc.sync.dma_start(w1c[:], w1_d[c])
                nc.scalar.dma_start(v1c[:], v1_d[c])
                w2src = w2q_d[c - (FCH - NQ8)] if fp8w2 else w2_d[c]
                if c == FCH - 1:
                    # split the final chunk's w2 into j-halves so the last
                    # down matmuls start before the full transfer lands
                    nc.sync.dma_start(w2c[:, 0 : 2 * H], w2src[:, 0 : 2 * H])
                    nc.scalar.dma_start(
                        w2c[:, 4 * H : 6 * H], w2src[:, 4 * H : 6 * H]
                    )
                    nc.sync.dma_start(w2c[:, 2 * H : 4 * H], w2src[:, 2 * H : 4 * H])
                    nc.scalar.dma_start(w2c[:, 6 * H :], w2src[:, 6 * H :])
                else:
                    nc.sync.dma_start(w2c[:, 0 : 4 * H], w2src[:, 0 : 4 * H])
                    nc.scalar.dma_start(w2c[:, 4 * H :], w2src[:, 4 * H :])
                wtiles[c] = (w1c, v1c, w2c, fp8w2)

            issue_chunk_dma(0)

            xg_sb = const_pool.tile([128, KT * T], bf16)
            nc.scalar.dma_start(xg_sb[:], xt_d[0])
            xu_sb = []
            for e in range(EPC):
                t_ = const_pool.tile([128, KT * T], bf16, tag=f"xu{e}")
                nc.scalar.dma_start(t_[:], xt_d[1 + e])
                xu_sb.append(t_)
            ident = const_pool.tile([128, 128], bf16)
            make_identity(nc, ident)

            # persistent down-projection accumulator:
            # [0:64, :] = hid 0..1023, [64:128, :] = hid 1024..2047
            down_ps = ps_down.tile([128, 1024], mybir.dt.float32)

            for c in range(FCH):
                if c not in wtiles:
                    issue_chunk_dma(c)
                w1c, v1c, w2c, fp8w2 = wtiles.pop(c)

                gate_ps = ps_gate.tile([128, FC], mybir.dt.float32, tag="gate")
                up_ps = ps_up.tile([128, FC], mybir.dt.float32, tag="up")
                # both experts share the xg stationary; alternate column
                # groups (out partitions 0-63 / 64-127) so the two M=64
                # matmuls run concurrently in the PE array.
                for i in range(KT):
                    for e in range(EPC):
                        nc.tensor.matmul(
                            gate_ps[64 * e : 64 * e + T, :],
                            xg_sb[:, i * T : (i + 1) * T],
                            w1c[:, e * EW + i * FC : e * EW + (i + 1) * FC],
                            start=(i == 0),
                            stop=(i == KT - 1),
                        )
                for i in range(KT):
                    for e in range(EPC):
                        nc.tensor.matmul(
                            up_ps[64 * e : 64 * e + T, :],
                            xu_sb[e][:, i * T : (i + 1) * T],
                            v1c[:, e * EW + i * FC : e * EW + (i + 1) * FC],
                            start=(i == 0),
                            stop=(i == KT - 1),
                        )

                gate_s = acts_pool.tile([128, FC], bf16, tag="gate_s")
                nc.scalar.activation(gate_s[:], gate_ps[:], act.Silu)
                h = acts_pool.tile([128, FC], bf16, tag="h")
                nc.vector.tensor_mul(h[:], gate_s[:], up_ps[:])

                # transpose h in full 128x128 blocks: block j becomes
                # [128 f, 128 t] with e0 tokens in cols 0-63, e1 in 64-127
                tp_ps = ps_tp.tile([128, 4 * 128], bf16, tag="tp")
                for j in range(4):
                    nc.tensor.transpose(
                        tp_ps[:, j * 128 : (j + 1) * 128],
                        h[:, j * 128 : (j + 1) * 128],
                        ident[:],
                    )
                hT = acts_pool.tile([128, 4 * 128], bf16, tag="hT")
                if fp8w2:
                    # fold the w2 fp8 scale in here: exact pow2 shift on bf16
                    nc.vector.tensor_scalar_mul(hT[:], tp_ps[:], k2inv)
                else:
                    nc.vector.tensor_copy(hT[:], tp_ps[:])

                def down(j, e):
                    st = hT[:, j * 128 + 64 * e : j * 128 + 64 * e + T]
                    wb = e * 4 * H + j * H
                    first = c == 0 and j == 0 and e == 0
                    last = c == FCH - 1 and j == 3 and e == EPC - 1
                    # hid quarters q0..q3; order (0,2,1,3) alternates
                    # column groups for PE concurrency
                    for q in (0, 2, 1, 3):
                        if q < 2:
                            dst = down_ps[0:T, q * 512 : (q + 1) * 512]
                        else:
                            dst = down_ps[64 : 64 + T, (q - 2) * 512 : (q - 1) * 512]
                        nc.tensor.matmul(
                            dst,
                            st,
                            w2c[:, wb + q * 512 : wb + (q + 1) * 512],
                            start=first,
                            stop=last,
                        )

                if c == FCH - 1:
                    # j-half order matching the split w2 DMAs
                    for jh in (0, 1):
                        for e in range(EPC):
                            for j in (2 * jh, 2 * jh + 1):
                                down(j, e)
                else:
                    for j in range(4):
                        for e in range(EPC):
                            down(j, e)

            # final drain in quarters, alternating engines and rings, so
            # each region streams out as soon as its accumulation stops
            out_sb = const_pool.tile([128, 1024], mybir.dt.float32)
            nc.vector.tensor_copy(out_sb[0:T, 0:512], down_ps[0:T, 0:512])
            nc.sync.dma_start(out_d[:, 0:512], out_sb[0:T, 0:512])
            nc.scalar.activation(
                out_sb[64 : 64 + T, 0:512], down_ps[64 : 64 + T, 0:512], act.Copy
            )
            nc.scalar.dma_start(out_d[:, 1024:1536], out_sb[64 : 64 + T, 0:512])
            nc.vector.tensor_copy(out_sb[0:T, 512:1024], down_ps[0:T, 512:1024])
            nc.sync.dma_start(out_d[:, 512:1024], out_sb[0:T, 512:1024])
            nc.scalar.activation(
                out_sb[64 : 64 + T, 512:1024],
                down_ps[64 : 64 + T, 512:1024],
                act.Copy,
            )
            nc.scalar.dma_start(out_d[:, 1536:2048], out_sb[64 : 64 + T, 512:1024])

    nc.compile()
    return nc


_NC_CACHE = {}


def _get_nc(k2):
    if k2 not in _NC_CACHE:
        _NC_CACHE[k2] = _build_nc(k2)
    return _NC_CACHE[k2]


def _swizzle_ffn(wt):
    """[H, F] (h, f) -> [FCH, 128, KT*FC] so chunk c is a contiguous
    [128, 8192] block with [p, i*FC + f] = wt[i*128 + p, c*FC + f]."""
    a = wt.reshape(KT, 128, FCH, FC)          # (i, p, c, f)
    return np.ascontiguousarray(a.transpose(2, 1, 0, 3)).reshape(FCH, 128, KT * FC)


def _swizzle_down(w2e):
    """[F, H] (f, hid) -> [FCH, 128, 4*H] so chunk c is contiguous
    [128, 8192] with [p, j*H + hid] = w2e[c*FC + j*128 + p, hid]."""
    a = w2e.reshape(FCH, 4, 128, H)           # (c, j, p, hid)
    return np.ascontiguousarray(a.transpose(0, 2, 1, 3)).reshape(FCH, 128, 4 * H)


def _pair(chunks):
    """stack 2 per-expert [FCH', 128, W] arrays -> [FCH', 128, 2*W]"""
    a = np.stack(chunks, axis=2)              # (c, p, e, w)
    c, p, e_, w = a.shape
    return np.ascontiguousarray(a).reshape(c, p, e_ * w)


def _pow2_scale(amax):
    return int(np.floor(np.log2(F8MAX * 0.98 / amax)))


def kernel(x, weights, top_weights, top_experts, w1, v1, w2):
    _ensure_axon_hooks()
    from concourse.bass_utils import run_bass_kernel_spmd

    x = np.asarray(x, dtype=np.float32).reshape(T, H)
    top_weights = np.asarray(top_weights, dtype=np.float32)
    top_experts = np.asarray(top_experts).astype(np.int64)
    w1 = np.asarray(w1, dtype=np.float32).reshape(E, F, H)
    v1 = np.asarray(v1, dtype=np.float32).reshape(E, F, H)
    w2 = np.asarray(w2, dtype=np.float32).reshape(E, F, H)

    # dense routing weights [T, E] (scatter-ADD: duplicate experts sum)
    r = np.zeros((T, E), np.float32)
    np.add.at(r, (np.arange(T)[:, None], top_experts), top_weights)

    # global power-of-2 scales for the fp8 weights; w1/v1 scales fold
    # exactly into the bf16 x operand planes, w2's into the hT drain
    k1 = _pow2_scale(np.abs(w1).max())
    kv = _pow2_scale(np.abs(v1).max())
    k2 = _pow2_scale(np.abs(w2).max())
    w1q = (w1 * np.float32(2.0**k1)).astype(F8E3)
    v1q = (v1 * np.float32(2.0**kv)).astype(F8E3)
    w2q = (w2 * np.float32(2.0**k2)).astype(F8E3)

    # x transposed/swizzled: [128, KT*T] with [p, i*T + t] = a[t, i*128 + p]
    def swz_x(a):
        return np.ascontiguousarray(
            a.T.reshape(KT, 128, T).transpose(1, 0, 2)
        ).reshape(128, KT * T).astype(BF16)

    xg = swz_x(x * np.float32(2.0**-k1))

    in_maps = []
    for core in range(N_CORES):
        es = [core * EPC + k for k in range(EPC)]
        w1t = _pair([_swizzle_ffn(w1q[e].T) for e in es])
        v1t = _pair([_swizzle_ffn(v1q[e].T) for e in es])
        w2bf = _pair([_swizzle_down(w2[e].astype(BF16))[: FCH - NQ8] for e in es])
        w2qs = _pair([_swizzle_down(w2q[e])[FCH - NQ8 :] for e in es])
        # plane 0: x*2^-k1 for the gate path; planes 1+k: r_e-scaled x
        # (times 2^-kv) for the up path
        xt_planes = np.stack(
            [xg]
            + [swz_x(x * (r[:, ee : ee + 1] * np.float32(2.0**-kv))) for ee in es],
            axis=0,
        )
        in_maps.append(
            {
                "xt": xt_planes,
                "w1t": w1t,
                "v1t": v1t,
                "w2s": w2bf,
                "w2q": w2qs,
            }
        )

    nc = _get_nc(k2)
    res = run_bass_kernel_spmd(nc, in_maps, core_ids=list(range(N_CORES)))
    out = np.zeros((T, H), np.float32)
    for c in range(N_CORES):
        out += res.results[c]["out"]
    return out.reshape(64, 1, H)
